# revision 27
# baseline (speedup 1.0000x reference)
"""Trainium2 Bass kernel for a pre-LN causal transformer block.

Design (v2):
  - All weights (Wq/Wk/Wv/Wo/W1/W2, LN folds, biases, identity, key-index
    table) are embedded in the NEFF as Const tensors via nc.inline_tensor:
    they are DMA'd to per-core DRAM once at model-load time, so the
    per-dispatch I/O is only the x shard + LN1 stats (~2 MB/core) and the
    2 MB/core output. No weight collectives.
  - Token sharding: pair of cores (2b, 2b+1) owns batch b. The even core
    takes 512-token blocks {0, 3}, the odd core blocks {1, 2}; this makes
    the causal work of the two cores near-equal, and the uniform SPMD
    attention loop only processes the union of needed (chunk, key-tile)
    pairs: 24 of 32 (the per-core mask data zeroes the tiles a given core
    doesn't need).
  - K/V cover all 2048 keys of the batch via a 2-core AllGather of the x
    shards (the only collective).
  - LayerNorms are folded into the projections (rank-2 correction matmul
    into the same PSUM group + per-token r multiply on the PSUM->SBUF
    copy). LN1 stats come from the host (tiny rows); LN2 stats are
    computed on device.

Matmul inputs are bf16 (full PE rate), accumulation f32 in PSUM, softmax
and residual f32.
"""

import math
import os
from contextlib import ExitStack

import numpy as np
import ml_dtypes

import concourse.bass as bass
import concourse.tile as tile
from concourse import bacc
from concourse.tile import add_dep_helper
from concourse import mybir
from concourse.bass_utils import run_bass_kernel_spmd

F32 = mybir.dt.float32
BF16 = mybir.dt.bfloat16
FP8 = mybir.dt.float8e4
AF = mybir.ActivationFunctionType
A = mybir.AluOpType

# Full-size problem dims (hardcoded; the harness provides x of this shape).
DIMS = dict(B=4, C=2048, E=1024, H=16, D=64, FF=4096, EPS=1e-5)
N_CORES = 8
P = 128

# token blocks: even core owns blocks (0, 3), odd core blocks (1, 2);
# xg key order = [even tokens | odd tokens]
BLK = 512
EVEN_BLOCKS = (0, 3)
ODD_BLOCKS = (1, 2)


def _key_positions(C):
    """Absolute positions of keys in xg slot order."""
    segs = [np.arange(b * BLK, (b + 1) * BLK)
            for b in (*EVEN_BLOCKS, *ODD_BLOCKS)]
    return np.concatenate(segs)


def _attn_schedule(C):
    """Uniform (chunk, key-slot) schedule.

    Returns per 512-query-chunk c: (slots, masked_slots) where slots is
    the list of key tiles the program processes and masked_slots those
    that get the per-core mask multiply (non-trivial for at least one
    core).
    """
    KT = C // P
    pos = _key_positions(C)
    kt_lo = [int(pos[kt * P:(kt + 1) * P].min()) for kt in range(KT)]
    kt_hi = [int(pos[kt * P:(kt + 1) * P].max()) for kt in range(KT)]

    # per-core query ranges per chunk
    core_chunks = {
        0: [(b * BLK, (b + 1) * BLK - 1) for b in EVEN_BLOCKS],
        1: [(b * BLK, (b + 1) * BLK - 1) for b in ODD_BLOCKS],
    }
    sched = []
    for c in range(2):
        slots, masked = [], []
        for kt in range(KT):
            needed = nontrivial = False
            for par in (0, 1):
                qlo, qhi = core_chunks[par][c]
                if kt_lo[kt] <= qhi:           # some key <= some query
                    needed = True
                    if kt_hi[kt] > qlo:        # not fully unmasked
                        nontrivial = True
                else:
                    nontrivial = True          # fully masked -> zeroing
            if needed:
                slots.append(kt)
                if nontrivial:
                    masked.append(kt)
        sched.append((slots, masked))
    return sched


def build_program(dims, weights):
    """Build the SPMD Bass program with weights baked in as consts."""
    B = dims["B"]
    C = dims["C"]
    E = dims["E"]
    H = dims["H"]
    D = dims["D"]
    FF = dims["FF"]
    EPS = dims["EPS"]

    TKV = C                      # kv tokens per core (full batch)
    TQ = B * C // N_CORES        # query tokens per core (1024)
    ES = E // P                  # E subtiles (contraction) = 8
    FS = FF // P                 # FF subtiles = 32
    HPAIRS = H // 2              # head pairs = 8
    NQUAD = H // 4               # head quads = 4
    KT = TKV // P                # key tiles = 16
    QC = TQ // 512               # 512-wide query chunks = 2
    NG = TKV // 1024             # 1024-token groups of kv = 2
    assert D == 64 and E == H * D

    bf = ml_dtypes.bfloat16
    f32 = np.float32

    nc = bacc.Bacc("TRN2", target_bir_lowering=False, debug=False,
                   num_devices=N_CORES)

    # ---- DRAM I/O (per-core, per-dispatch) ----
    xq_d = nc.dram_tensor("xq", [E, TQ], BF16, kind="ExternalInput")
    st_d = nc.dram_tensor("stats", [4, TQ], F32, kind="ExternalInput")
    sk_d = nc.dram_tensor("statsk", [4, TKV], F32, kind="ExternalInput")
    out_d = nc.dram_tensor("outT", [E, TQ], BF16, kind="ExternalOutput")

    # ---- consts baked into the NEFF ----
    g1 = np.asarray(weights["ln1_g"], f32)
    b1_ = np.asarray(weights["ln1_b"], f32)
    sc = 1.0 / math.sqrt(D)
    wq = g1[:, None] * np.asarray(weights["Wq"], f32) * sc
    wk = g1[:, None] * np.asarray(weights["Wk"], f32)
    wv = g1[:, None] * np.asarray(weights["Wv"], f32)
    wo = np.asarray(weights["Wo"], f32)
    w1 = np.asarray(weights["ln2_g"], f32)[:, None] * \
        np.asarray(weights["W1"], f32)
    w2 = np.asarray(weights["W2"], f32)
    b1f = (np.asarray(weights["b1"], f32)
           + np.asarray(weights["ln2_b"], f32) @ np.asarray(weights["W1"],
                                                            f32))

    wq_c = nc.inline_tensor(wq.astype(bf), name="wq_c")
    wk_c = nc.inline_tensor(wk.astype(bf), name="wk_c")
    wv_c = nc.inline_tensor(wv.astype(bf), name="wv_c")
    wo_c = nc.inline_tensor(wo.astype(bf), name="wo_c")
    w1_c = nc.inline_tensor(w1.astype(bf), name="w1_c")
    w2_c = nc.inline_tensor(w2.astype(bf), name="w2_c")

    foldqkv = np.zeros((P, 3 * E), f32)
    foldqkv[0, 0:E] = wq.sum(0)
    foldqkv[0, E:2 * E] = wk.sum(0)
    foldqkv[0, 2 * E:3 * E] = wv.sum(0)
    foldqkv[1, 0:E] = b1_ @ wq
    foldqkv[1, E:2 * E] = b1_ @ wk
    foldqkv[1, 2 * E:3 * E] = b1_ @ wv
    foldqkv_c = nc.inline_tensor(foldqkv.astype(bf), name="foldqkv_c")

    w1f = np.zeros((P, FF), f32)
    w1f[0, :] = w1.sum(0)
    w1f_c = nc.inline_tensor(w1f.astype(bf), name="w1f_c")

    kpos = _key_positions(C)
    kidx = kpos.reshape(KT, P).T.astype(f32)          # [P, KT]
    kidx_c = nc.inline_tensor(kidx, name="kidx_c")

    ident_c = nc.inline_tensor(np.eye(P, dtype=f32).astype(bf),
                               name="ident_c")
    b1f_c = nc.inline_tensor(
        np.ascontiguousarray(b1f.reshape(FS, P).T).astype(f32),
        name="b1f_c")
    b2f_c = nc.inline_tensor(
        np.ascontiguousarray(
            np.asarray(weights["b2"], f32).reshape(ES, P).T),
        name="b2f_c")

    wq3 = wq_c.rearrange("(s p) n -> p s n", p=P)
    wk3 = wk_c.rearrange("(s p) n -> p s n", p=P)
    wv3 = wv_c.rearrange("(s p) n -> p s n", p=P)
    wo3 = wo_c.rearrange("(s p) e -> p s e", p=P)
    w13 = w1_c.rearrange("(s p) f -> p s f", p=P)
    w23 = w2_c.rearrange("(s p) e -> p s e", p=P)

    xq3 = xq_d.rearrange("(s p) t -> p s t", p=P)
    out3 = out_d.rearrange("(s p) t -> p s t", p=P)
    PAIRS = [[2 * i, 2 * i + 1] for i in range(N_CORES // 2)]

    SCHED = _attn_schedule(C)

    with tile.TileContext(nc) as tc, ExitStack() as ctx:
        perm = ctx.enter_context(tc.tile_pool(name="perm", bufs=1))
        tmp = ctx.enter_context(tc.tile_pool(name="tmp", bufs=2))
        wstream = ctx.enter_context(tc.tile_pool(name="wstream", bufs=3))
        ps = ctx.enter_context(tc.tile_pool(name="ps", bufs=2, space="PSUM"))
        pso = ctx.enter_context(tc.tile_pool(name="pso", bufs=2, space="PSUM"))
        dpool = ctx.enter_context(tc.tile_pool(name="dpool", bufs=2,
                                               space="DRAM"))
        dgath = ctx.enter_context(tc.tile_pool(name="dgath", bufs=1,
                                               space="DRAM"))

        # ---------- collective: gather pair x halves ----------
        xb = dgath.tile([E, TQ], BF16, tag="xb")
        xg = dgath.tile([2, E, TQ], BF16, tag="xg")
        nc.gpsimd.dma_start(xb[:], xq_d[:, :])
        nc.gpsimd.collective_compute(
            "AllGather", A.bypass, replica_groups=PAIRS,
            ins=[xb.opt()], outs=[xg.opt()])

        def bcast_rows(dst, srcrow, nrows, width):
            """Broadcast a [1, width] sbuf row to [nrows, width] via DRAM."""
            row_d = dpool.tile([1, width], srcrow.dtype, tag="row_d")
            nc.sync.dma_start(row_d, srcrow)
            bsrc = bass.AP(tensor=row_d.tensor, offset=row_d.offset,
                           ap=[[0, nrows]] + row_d.ap[1:])
            nc.gpsimd.dma_start(dst, bsrc)

        def bcast_dram_row(dst, src_ap, nrows, width, dtype):
            """Broadcast a [1, width] DRAM row to [nrows, width] sbuf."""
            row_d = dpool.tile([1, width], dtype, tag="row_d")
            nc.sync.dma_start(row_d, src_ap)
            bsrc = bass.AP(tensor=row_d.tensor, offset=row_d.offset,
                           ap=[[0, nrows]] + row_d.ap[1:])
            nc.gpsimd.dma_start(dst, bsrc)

        ones_bf = perm.tile([P, 1], BF16, tag="ones_bf")
        nc.vector.memset(ones_bf, 1.0)

        # ACT LUT table management: Exp and Gelu live in different hardware
        # tables; emit zero-dependency dummy activations to carry each
        # switch, ordered on the ACT stream with same-engine dep edges.
        scr_in = perm.tile([1, 8], F32, tag="scr_in")
        nc.vector.memset(scr_in, 1.0)
        scr_out = perm.tile([1, 8], F32, tag="scr_out")
        dummy_exp = nc.scalar.activation(scr_out, scr_in, AF.Exp)
        act_exp_insts = []

        # Warm up engine opcodes on scratch so first-use config loads
        # don't ride real (multi-wait) instructions.
        nc.vector.tensor_copy(scr_out, scr_in)
        nc.vector.tensor_mul(scr_out, scr_in, scr_in)
        nc.vector.tensor_add(scr_out, scr_in, scr_in)
        nc.vector.tensor_sub(scr_out, scr_in, scr_in)
        nc.vector.tensor_scalar(scr_out, scr_in, 0.5, 0.5, A.mult, A.add)
        nc.vector.tensor_scalar(scr_out, scr_in, 0.5, None, A.is_ge)
        nc.vector.tensor_scalar_mul(scr_out, scr_in, 0.5)
        nc.vector.tensor_scalar_add(scr_out, scr_in, 0.5)
        nc.vector.reciprocal(scr_out, scr_in)
        nc.vector.scalar_tensor_tensor(scr_out, scr_in, 0.5, scr_in,
                                       A.add, A.add)
        scr_bf = perm.tile([1, 8], BF16, tag="scr_bf")
        nc.vector.memset(scr_bf, 1.0)
        nc.vector.tensor_mul(scr_bf, scr_bf, scr_bf)
        scr_ps = ps.tile([P, 1024], F32, tag="ps")
        nc.tensor.matmul(scr_ps[0:8, 0:8], scr_bf[0:1, 0:8],
                         scr_bf[0:1, 0:8], start=True, stop=True)
        nc.vector.tensor_copy(scr_out, scr_ps[0:1, 0:8])

        b1f_sb = perm.tile([P, FS], F32, tag="b1f")
        nc.sync.dma_start(b1f_sb, b1f_c[:, :])
        b2f_sb = perm.tile([P, ES], F32, tag="b2f")
        nc.sync.dma_start(b2f_sb, b2f_c[:, :])
        hidden = perm.tile([P, HPAIRS, TQ], BF16, tag="hidden")

        with tc.tile_pool(name="att", bufs=1) as att:
            ident_sb = att.tile([P, P], BF16, tag="ident")
            nc.sync.dma_start(ident_sb, ident_c[:, :])
            kidx_sb = att.tile([P, KT], F32, tag="kidx")
            nc.sync.dma_start(kidx_sb, kidx_c[:, :])
            # LN1 fold rows (host stats): row0=-mu, row1=sd, rows 2..=0
            foldrowQ = att.tile([P, TQ], BF16, tag="frQ")
            nc.vector.memset(foldrowQ, 0.0)
            nc.gpsimd.dma_start(foldrowQ[0:1, :], st_d[0:1, :])
            nc.gpsimd.dma_start(foldrowQ[1:2, :], st_d[1:2, :])
            foldrowK = att.tile([P, TKV], BF16, tag="frK")
            nc.vector.memset(foldrowK, 0.0)
            nc.gpsimd.dma_start(foldrowK[0:1, :], sk_d[0:1, :])
            nc.gpsimd.dma_start(foldrowK[1:2, :], sk_d[1:2, :])
            # r broadcast tiles (f32)
            rbcQ = att.tile([P, TQ], F32, tag="rbcQ")
            bcast_dram_row(rbcQ, st_d[2:3, :], P, TQ, F32)
            rbcK = att.tile([P, TKV], F32, tag="rbcK")
            bcast_dram_row(rbcK, sk_d[2:3, :], P, TKV, F32)
            # fold stationary for q/k/v: row0 = colsum(W'), row1 = beta@W'
            foldWqkv = att.tile([P, 3 * E], BF16, tag="foldWqkv")
            nc.sync.dma_start(foldWqkv, foldqkv_c[:, :])
            # causal mask, only for the (chunk, slot) combos that need it:
            # mask2[p, i, 0:512] = mask2[p, i, 512:1024]
            #   = (qoff[chunk q] >= kidx[p, slot])
            MASKED_COMBOS = []
            for cq in range(QC):
                for kt in SCHED[cq][1]:
                    MASKED_COMBOS.append((cq, kt))
            MIDX = {ck: i for i, ck in enumerate(MASKED_COMBOS)}
            mask_sb = att.tile([P, len(MASKED_COMBOS), 1024], FP8,
                               tag="mask")
            with tc.tile_pool(name="maskbuild", bufs=1) as mb:
                # q-position broadcast (f32), duplicated per 512-chunk so
                # one [P, 1024] mask row covers both heads' halves of the
                # paired score tile
                qb2 = mb.tile([P, QC, 1024], F32, tag="qb2")
                for cq in range(QC):
                    for half in range(2):
                        bcast_dram_row(
                            qb2[:, cq, half * 512:(half + 1) * 512],
                            st_d[3:4, cq * 512:(cq + 1) * 512], P, 512, F32)
                for (cq, kt), i in MIDX.items():
                    nc.vector.tensor_scalar(mask_sb[:, i], qb2[:, cq],
                                            kidx_sb[:, kt:kt + 1], None,
                                            A.is_ge)

            # gathered x (E-major, both halves of the batch)
            xsb = att.tile([P, ES, TKV], BF16, tag="xsb")
            for h in range(2):
                nc.sync.dma_start(
                    xsb[:, :, h * TQ:(h + 1) * TQ],
                    xg[h].rearrange("(s p) t -> p s t", p=P))

            def project(dst_ap, wview, fold_off, moving, frow, rbc, ntok,
                        m):
                """One 128-col block of a folded-LN projection.

                dst_ap: [P, ntok] bf16 destination (r-mult applied)
                wview:  [p, s, n] DRAM view of W' (E-contraction)
                fold_off: column offset of this projection in foldWqkv
                moving: [P, ES, ntok] bf16 sbuf
                """
                wt = wstream.tile([P, ES, P], BF16, tag="w")
                nc.sync.dma_start(wt, wview[:, :, m * P:(m + 1) * P])
                for g in range(ntok // 1024):
                    pst = ps.tile([P, 1024], F32, tag="ps")
                    for c in range(2):
                        tsl = slice(g * 1024 + c * 512,
                                    g * 1024 + (c + 1) * 512)
                        psl = pst[:, c * 512:(c + 1) * 512]
                        for s in range(ES):
                            nc.tensor.matmul(psl, wt[:, s], moving[:, s, tsl],
                                             start=(s == 0), stop=False)
                        nc.tensor.matmul(
                            psl,
                            foldWqkv[:, fold_off + m * P:fold_off + (m + 1) * P],
                            frow[:, tsl], start=False, stop=True)
                    gsl = slice(g * 1024, (g + 1) * 1024)
                    nc.vector.tensor_tensor(dst_ap[:, gsl], pst[:, 0:1024],
                                            rbc[:, gsl], A.mult)

            # Q for own tokens (all head pairs)
            qt_all = att.tile([P, HPAIRS, TQ], BF16, tag="qt_all")
            with tc.tile_pool(name="proj", bufs=1) as proj:
                xqsb = proj.tile([P, ES, TQ], BF16, tag="xqsb")
                nc.sync.dma_start(xqsb, xq3[:, :, :])
                for m in range(HPAIRS):
                    project(qt_all[:, m], wq3, 0, xqsb, foldrowQ, rbcQ,
                            TQ, m)

            # K/V per quad + attention
            if True:
                with tc.tile_pool(name="quad", bufs=2) as quad, \
                     tc.tile_pool(name="ppool", bufs=5) as ppool:
                    for q4 in range(NQUAD):
                        vq = quad.tile([P, KT, 4 * 65], BF16, tag="vq")
                        nc.vector.memset(vq, 1.0)
                        vq_v = vq.rearrange("p t (h c) -> p t h c", c=65)
                        kts = []
                        for pr2 in range(2):
                            m = q4 * 2 + pr2
                            ktp = quad.tile([P, TKV], BF16, tag="ktp")
                            project(ktp, wk3, E, xsb, foldrowK, rbcK,
                                    TKV, m)
                            kts.append(ktp)
                            # V block + transpose into vq
                            wtv = wstream.tile([P, ES, P], BF16, tag="w")
                            nc.sync.dma_start(wtv,
                                              wv3[:, :, m * P:(m + 1) * P])
                            for g in range(NG):
                                pst = ps.tile([P, 1024], F32, tag="ps")
                                for c in range(2):
                                    tsl = slice(g * 1024 + c * 512,
                                                g * 1024 + (c + 1) * 512)
                                    psl = pst[:, c * 512:(c + 1) * 512]
                                    for s in range(ES):
                                        nc.tensor.matmul(
                                            psl, wtv[:, s], xsb[:, s, tsl],
                                            start=(s == 0), stop=False)
                                    nc.tensor.matmul(
                                        psl,
                                        foldWqkv[:, 2 * E + m * P:
                                                 2 * E + (m + 1) * P],
                                        foldrowK[:, tsl],
                                        start=False, stop=True)
                                vtmp = tmp.tile([P, 1024], BF16, tag="vtmp")
                                gsl = slice(g * 1024, (g + 1) * 1024)
                                nc.vector.tensor_tensor(
                                    vtmp, pst[:, 0:1024], rbcK[:, gsl],
                                    A.mult)
                                pst2 = ps.tile([P, 1024], F32, tag="ps")
                                for j in range(8):
                                    kt = g * 8 + j
                                    jsl = slice(j * P, (j + 1) * P)
                                    # transpose: out[i,j] = sum_p v[p,i]*I[p,j]
                                    nc.tensor.matmul(
                                        pst2[:, jsl], vtmp[:, jsl], ident_sb,
                                        start=True, stop=True)
                                    nc.vector.tensor_copy(
                                        vq_v[:, kt, 2 * pr2:2 * pr2 + 2,
                                             0:64],
                                        pst2[:, j * P:(j + 1) * P])

                        # attention for the quad's two pairs
                        for pr2 in range(2):
                            m = q4 * 2 + pr2
                            ktp = kts[pr2]
                            opsA = pso.tile([65, 1024], F32, tag="opsum")
                            opsB = pso.tile([65, 1024], F32, tag="opsum")
                            for cq in range(QC):
                                qsl = slice(cq * 512, (cq + 1) * 512)
                                slots, masked = SCHED[cq]
                                for si, kt in enumerate(slots):
                                    ksl = slice(kt * P, (kt + 1) * P)
                                    # both heads' scores into one [P, 1024]
                                    # psum: cols 0:512 = hh0, 512: = hh1
                                    sct = ps.tile([P, 1024], F32, tag="ps")
                                    for hh in range(2):
                                        rows = slice(hh * 64, hh * 64 + 64)
                                        nc.tensor.matmul(
                                            sct[:, hh * 512:(hh + 1) * 512],
                                            ktp[rows, ksl],
                                            qt_all[rows, m, qsl],
                                            start=True, stop=True)
                                    pt = ppool.tile([P, 1024], BF16,
                                                    tag="pT")
                                    _ei = nc.scalar.activation(
                                        pt, sct, AF.Exp)
                                    act_exp_insts.append(_ei)
                                    add_dep_helper(
                                        _ei.ins, dummy_exp.ins,
                                        sync=True,
                                        reason="act table: exp")
                                    if kt in masked:
                                        nc.vector.tensor_tensor(
                                            pt, pt,
                                            mask_sb[:, MIDX[(cq, kt)]],
                                            A.mult)
                                    for hh, ops in ((0, opsA), (1, opsB)):
                                        h4 = 2 * pr2 + hh
                                        vcols = slice(h4 * 65, h4 * 65 + 65)
                                        nc.tensor.matmul(
                                            ops[:, qsl],
                                            vq[:, kt, vcols],
                                            pt[:, hh * 512:(hh + 1) * 512],
                                            start=(si == 0),
                                            stop=(si == len(slots) - 1))
                            # normalize: hidden = O / sum (sum at row 64)
                            for hh, ops in ((0, opsA), (1, opsB)):
                                ssb = tmp.tile([65, TQ], F32, tag="ssb")
                                nc.vector.reciprocal(ssb[64:65],
                                                     ops[64:65, 0:TQ])
                                rb = tmp.tile([64, TQ], F32, tag="t4")
                                bcast_rows(rb, ssb[64:65, :], 64, TQ)
                                if hh == 0:
                                    nc.vector.tensor_tensor(
                                        hidden[0:64, m], ops[0:64, 0:TQ],
                                        rb, A.mult)
                                else:
                                    hb = tmp.tile([64, TQ], BF16, tag="hb")
                                    nc.vector.tensor_tensor(
                                        hb, ops[0:64, 0:TQ], rb, A.mult)
                                    nc.gpsimd.dma_start(hidden[64:128, m],
                                                        hb)

        # ---------- Wo + residual, LN2, FFN ----------
        # (ln_stats computes per-token mean/rstd over E via ones-matmul)
        def ln_stats(src_sb, ntok, foldrow, a_bcast):
            nchunk = ntok // 512
            w = 512
            for c in range(nchunk):
                sl = slice(c * w, (c + 1) * w)
                pst = ps.tile([P, 1024], F32, tag="ps")
                psum_s = pst[0:1, 0:w]
                psum_q = pst[0:1, 512:512 + w]
                for s in range(ES):
                    nc.tensor.matmul(psum_s, ones_bf, src_sb[:, s, sl],
                                     start=(s == 0), stop=(s == ES - 1))
                for s in range(ES):
                    sq_s = tmp.tile([P, w], BF16, tag="sq_s")
                    nc.vector.tensor_mul(sq_s, src_sb[:, s, sl],
                                         src_sb[:, s, sl])
                    nc.tensor.matmul(psum_q, ones_bf, sq_s,
                                     start=(s == 0), stop=(s == ES - 1))
                mu = tmp.tile([1, w], F32, tag="mu")
                nc.vector.tensor_scalar_mul(mu, psum_s, 1.0 / E)
                m2 = tmp.tile([1, w], F32, tag="m2")
                nc.vector.tensor_scalar_mul(m2, psum_q, 1.0 / E)
                var = tmp.tile([1, w], F32, tag="var")
                nc.vector.tensor_mul(var, mu, mu)
                nc.vector.tensor_sub(var, m2, var)
                nc.vector.tensor_scalar_add(var, var, EPS)
                # r = rsqrt(var) via reciprocal seed + 3 Newton steps
                w_ = tmp.tile([1, w], F32, tag="wrec")
                nc.vector.reciprocal(w_, var)
                r_ = tmp.tile([1, w], F32, tag="rr")
                nc.vector.tensor_scalar(r_, w_, 0.5, 0.5, A.mult, A.add)
                t_ = tmp.tile([1, w], F32, tag="tt")
                for _ in range(3):
                    nc.vector.tensor_mul(t_, r_, r_)
                    nc.vector.tensor_mul(t_, t_, var)
                    nc.vector.tensor_scalar(t_, t_, -0.5, 1.5,
                                            A.mult, A.add)
                    nc.vector.tensor_mul(r_, r_, t_)
                nc.vector.tensor_copy(a_bcast[0:1, sl], r_)
                # fold row0 = -mu*r (the W1 moving operand is pre-scaled
                # by r, so the mean correction scales too)
                nmr = tmp.tile([1, w], F32, tag="nmr")
                nc.vector.tensor_mul(nmr, mu, r_)
                nc.vector.tensor_scalar_mul(foldrow[0:1, sl], nmr, -1.0)
            bcast_rows(a_bcast[1:P, :], a_bcast[0:1, :], P - 1, ntok)

        with tc.tile_pool(name="post", bufs=1) as post:
            out1bf = post.tile([P, ES, TQ], BF16, tag="out1bf")
            out1s = post.tile([P, ES, TQ], BF16, tag="out1s")
            for et in range(ES):
                wo_et = wstream.tile([P, ES, P], BF16, tag="w")
                nc.sync.dma_start(wo_et, wo3[:, :, et * P:(et + 1) * P])
                pst = ps.tile([P, 1024], F32, tag="ps")
                for c in range(QC):
                    psl = pst[:, c * 512:(c + 1) * 512]
                    qsl = slice(c * 512, (c + 1) * 512)
                    for s in range(ES):
                        nc.tensor.matmul(psl, wo_et[:, s], hidden[:, s, qsl],
                                         start=(s == 0), stop=(s == ES - 1))
                xr = tmp.tile([P, TQ], BF16, tag="xr")
                nc.sync.dma_start(xr, xq3[:, et])
                nc.vector.tensor_add(out1bf[:, et], pst[:, 0:TQ], xr)

            foldrow2 = post.tile([P, TQ], BF16, tag="foldrow2")
            nc.vector.memset(foldrow2, 0.0)
            a2 = post.tile([P, TQ], BF16, tag="a2")
            ln_stats(out1bf, TQ, foldrow2, a2)
            # W1 moving operand pre-scaled by r2 so gelu can read the
            # matmul PSUM directly (no per-token multiply on DVE)
            for s in range(ES):
                nc.vector.tensor_tensor(out1s[:, s], out1bf[:, s], a2,
                                        A.mult)

            scr_out2 = perm.tile([1, 8], F32, tag="scr_out2")
            dummy_gelu = nc.scalar.activation(scr_out2, scr_in, AF.Gelu)
            for ei in act_exp_insts:
                add_dep_helper(dummy_gelu.ins, ei.ins, sync=True,
                               reason="act table: gelu after all exps")

            # w1 fold stationary: row0 = colsum(W1'), rows 1.. = 0
            w1f_sb = post.tile([P, FF], BF16, tag="w1f")
            nc.sync.dma_start(w1f_sb, w1f_c[:, :])

            h3 = post.tile([P, FS, TQ], BF16, tag="h3")
            for ft in range(FS):
                w1_ft = wstream.tile([P, ES, P], BF16, tag="w")
                nc.sync.dma_start(w1_ft, w13[:, :, ft * P:(ft + 1) * P])
                pst = ps.tile([P, 1024], F32, tag="ps")
                for c in range(QC):
                    psl = pst[:, c * 512:(c + 1) * 512]
                    qsl = slice(c * 512, (c + 1) * 512)
                    for s in range(ES):
                        nc.tensor.matmul(psl, w1_ft[:, s], out1s[:, s, qsl],
                                         start=(s == 0), stop=False)
                    nc.tensor.matmul(psl, w1f_sb[:, ft * P:(ft + 1) * P],
                                     foldrow2[:, qsl], start=False, stop=True)
                gi = nc.scalar.activation(h3[:, ft], pst[:, 0:TQ], AF.Gelu,
                                          bias=b1f_sb[:, ft:ft + 1])
                add_dep_helper(gi.ins, dummy_gelu.ins, sync=True,
                               reason="act table: gelu after switch")

            for et in range(ES):
                pst = ps.tile([P, 1024], F32, tag="ps")
                for part in range(FS // ES):
                    w2_et = wstream.tile([P, ES, P], BF16, tag="w")
                    nc.sync.dma_start(
                        w2_et,
                        w23[:, part * ES:(part + 1) * ES,
                            et * P:(et + 1) * P])
                    for c in range(QC):
                        psl = pst[:, c * 512:(c + 1) * 512]
                        qsl = slice(c * 512, (c + 1) * 512)
                        for s8 in range(ES):
                            s = part * ES + s8
                            nc.tensor.matmul(psl, w2_et[:, s8],
                                             h3[:, s, qsl],
                                             start=(s == 0),
                                             stop=(s == FS - 1))
                ot = tmp.tile([P, TQ], BF16, tag="ot")
                nc.vector.scalar_tensor_tensor(
                    ot, pst[:, 0:TQ], b2f_sb[:, et:et + 1], out1bf[:, et],
                    A.add, A.add)
                nc.sync.dma_start(out3[:, et], ot)

    nc.compile()
    return nc


# ---------------------------------------------------------------------------
# Host side
# ---------------------------------------------------------------------------

def _core_token_idx(core):
    blocks = EVEN_BLOCKS if core % 2 == 0 else ODD_BLOCKS
    return np.concatenate([np.arange(b * BLK, (b + 1) * BLK)
                           for b in blocks])


def prep_inputs(dims, x, **_unused):
    """Build per-core in_maps (x shard + LN1 stats only)."""
    B, C, E = dims["B"], dims["C"], dims["E"]
    EPS = dims["EPS"]
    TQ = B * C // N_CORES
    bf = ml_dtypes.bfloat16
    f32 = np.float32

    x = np.asarray(x, f32)
    mu = x.mean(-1)                            # [B, C]
    var = x.var(-1)
    sd = np.sqrt(var + EPS)

    kpos = _key_positions(C)

    in_maps = []
    for c in range(N_CORES):
        b = c // 2
        tok = _core_token_idx(c)
        stats = np.zeros((4, TQ), f32)
        stats[0] = -mu[b, tok]
        stats[1] = sd[b, tok]
        stats[2] = 1.0 / sd[b, tok]
        stats[3] = tok.astype(f32)
        statsk = np.zeros((4, C), f32)
        statsk[0] = -mu[b, kpos]
        statsk[1] = sd[b, kpos]
        statsk[2] = 1.0 / sd[b, kpos]
        m = {
            "xq": np.ascontiguousarray(x[b, tok].T).astype(bf),
            "stats": stats,
            "statsk": statsk,
        }
        in_maps.append(m)
    return in_maps


def assemble_output(dims, results):
    B, C, E = dims["B"], dims["C"], dims["E"]
    out = np.empty((B, C, E), np.float32)
    for c in range(N_CORES):
        b = c // 2
        tok = _core_token_idx(c)
        out[b, tok] = np.asarray(
            results[c]["outT"], dtype=np.float32).T
    return out


def kernel(**inputs):
    dims = DIMS
    inputs = {k: np.asarray(v) for k, v in inputs.items()}
    nc = build_program(dims, inputs)
    in_maps = prep_inputs(dims, **inputs)
    res = run_bass_kernel_spmd(nc, in_maps, list(range(N_CORES)))
    return assemble_output(dims, res.results)


if __name__ == "__main__":
    pass
